# revision 1
# baseline (speedup 1.0000x reference)
"""Trainium2 Bass kernel for nn_GCK3x3Layer: 3x3 VALID conv, 256->256 ch, 258x258.

result = kernelsL @ im2col_3x3(input); input (1,256,258,258) f32,
kernelsL (256, 2304) f32 -> output (1, 256, 256, 256) f32.

Strategy: spatial-parallel across 8 NeuronCores. Each core gets a 34-row
input slab (32 output rows + 2 halo rows) and the full weight matrix, and
computes all 256 output channels for its strip via implicit-GEMM:
for each of 9 filter taps and 2 input-channel blocks, a [128,128]x[128,512]
matmul accumulating into PSUM (K = 2304 contraction in 18 chunks of 128,
N = 512 = two output rows of 256 pixels).
"""

import os
import sys
from contextlib import ExitStack

import numpy as np

for _p in (
    "/root/.axon_site",
    "/root/.axon_site/_ro/trn_rl_repo",
    "/root/.axon_site/_ro/pypackages",
    "/opt/trn_rl_repo",
):
    if os.path.isdir(_p) and _p not in sys.path:
        sys.path.append(_p)

import ml_dtypes  # noqa: E402

import concourse.bass as bass  # noqa: E402
import concourse.tile as tile  # noqa: E402
from concourse import bacc, mybir  # noqa: E402
from concourse.bass_utils import run_bass_kernel_spmd  # noqa: E402

IN_C = 256
OUT_C = 256
H = 258
W = 258
H_OUT = H - 2  # 256
W_OUT = W - 2  # 256
NCORES = 8
ROWS_PER_CORE = H_OUT // NCORES  # 32
IN_ROWS = ROWS_PER_CORE + 2  # 34
P = 128
ICB = IN_C // P  # 2 input-channel blocks
OCB = OUT_C // P  # 2 output-channel blocks
KB = ICB * 9  # 18 contraction blocks of 128
PAIRS = ROWS_PER_CORE // 2  # 16 output-row pairs (N=512 per matmul)

F32 = mybir.dt.float32


def build(
    mm_dtype=mybir.dt.bfloat16,
    repeat=1,
    x_chunk_rows=6,
    loop_repeat=1,
    out_dt=mybir.dt.bfloat16,
    split_queues=True,
    same_weights=False,  # TIMING PROBE ONLY: reuse one weight tile in all
    # matmuls (wrong numerics) to see if repeated identical LDWEIGHTS get
    # elided / hidden. Never used by kernel().
    rows_per_mm=2,  # output rows per matmul: 2 -> N=512 (one PSUM bank),
    # 4 -> N=1024 (PSUM tile spans two banks, halves matmul count).
    # NOTE: 4 is rejected by the ISA (s3d3_mm_num_elements) - matmul
    # output must fit one PSUM bank. Keep 2.
    skip_out=False,  # TIMING PROBE ONLY: drop PSUM->SBUF copies and
    # output stores (wrong output) to bracket the drain-path cost.
    interleave=False,  # interleave the two ocb accumulation groups of each
    # row-pair (two PSUM banks in flight), halving group boundaries so the
    # PE issue stream has fewer chances to micro-idle (HAM oscillation).
):
    """Build + compile the per-core Bass program (identical on all cores).

    mm_dtype: matmul operand dtype. bfloat16 (default) halves DMA/SBUF
    traffic and enables the compiler's fast-weight-load path (FWL is
    disabled for 4-byte operands), hiding LDWEIGHTS behind streaming.
    Accuracy vs the f32 reference is ~2.7e-3 relmax (quantization of both
    operands, fp32 PSUM accumulation), measured offline on the exact
    problem data.
    out_dt: y DMA dtype. bfloat16 halves store traffic (host upcasts);
    adds ~<1e-3 to relmax error.
    split_queues: issue y stores on the ACT HWDGE queue instead of SP, so
    next iteration's x prefetch (SP queue) isn't FIFO-blocked behind this
    iteration's 32 output stores.
    repeat: python-unrolled repetitions of the compute pass (dev timing).
    loop_repeat: hardware For_i repetitions of the whole pass (dev timing).
    """
    nc = bacc.Bacc(
        "TRN2", target_bir_lowering=False, debug=False, num_devices=NCORES
    )
    in_dt = F32 if mm_dtype == mybir.dt.float32r else mm_dtype
    nrep = None
    if loop_repeat == "dynamic":
        # Runtime-controlled repeat count (timing harness): one NEFF serves
        # every rep count. Loaded straight from DRAM into per-engine regs,
        # same mechanism as partition_id.
        nrep = nc.dram_tensor(
            "nrep", [1, 1], mybir.dt.uint32, kind="ExternalInput"
        )
    x = nc.dram_tensor("x", [IN_C, IN_ROWS * W], in_dt, kind="ExternalInput")
    wT = nc.dram_tensor("wT", [9 * IN_C, OUT_C], in_dt, kind="ExternalInput")
    y = nc.dram_tensor(
        "y", [OUT_C, ROWS_PER_CORE * W_OUT], out_dt, kind="ExternalOutput"
    )

    xv = x.rearrange("(b p) (r c) -> p b r c", p=P, c=W)
    wv = wT.rearrange("(b p) m -> p b m", p=P)
    if mm_dtype == mybir.dt.float32r:
        # f32r is bit-compatible with f32; declaring the SBUF tiles f32r
        # (and bitcasting the DMA source) satisfies the walrus requirement
        # that FP32r matmul operands come from an f32r-typed producer.
        xv = xv.bitcast(mm_dtype)
        wv = wv.bitcast(mm_dtype)

    looped = loop_repeat == "dynamic" or loop_repeat > 1
    with tile.TileContext(nc) as tc:
        with ExitStack() as ctx:
            xpool = ctx.enter_context(
                tc.tile_pool(name="xp", bufs=2 if looped else 1)
            )
            wpool = ctx.enter_context(tc.tile_pool(name="wp", bufs=1))
            pspool = ctx.enter_context(
                tc.tile_pool(
                    name="ps", bufs=16 // rows_per_mm, space="PSUM"
                )
            )
            opool = ctx.enter_context(tc.tile_pool(name="op", bufs=4))

            # HAM warmup: the PE clock is gated to 1.2 GHz until ~3.4us of
            # sustained activity. Fill the initial DMA wait (weights + first
            # input chunk) with throwaway fp32 matmuls on a zeroed tile so
            # the real f32r stream starts at the full 2.4 GHz. fp32 avoids
            # the f32r rounded-producer requirement; results are never read.
            warm = wpool.tile([P, P], F32, name="warm")
            nc.gpsimd.memset(warm[:], 0.0)
            wps = pspool.tile([P, rows_per_mm, W_OUT], F32, name="ps", tag="ps")
            for _ in range(12):
                nc.tensor.matmul(
                    wps[:, 0, 0:P],
                    warm[:],
                    warm[:],
                    start=True,
                    stop=True,
                    skip_group_check=True,
                )

            # Split the weight load by out-channel half: the first
            # accumulation group only consumes ocb=0 columns, so compute can
            # start once the first half (~1.2MB) lands instead of waiting for
            # the full 2.3MB transfer; the ocb=1 half streams in behind it.
            w_sb = wpool.tile([P, KB, OUT_C], mm_dtype)
            nc.sync.dma_start(w_sb[:, :, 0:P], wv[:, :, 0:P])
            nc.sync.dma_start(w_sb[:, :, P:OUT_C], wv[:, :, P:OUT_C])

            def _one_pass():
                x_sb = xpool.tile([P, ICB, IN_ROWS, W], mm_dtype, name="x_sb")
                r0 = 0
                while r0 < IN_ROWS:
                    r1 = min(r0 + x_chunk_rows, IN_ROWS)
                    for b in range(ICB):
                        nc.sync.dma_start(
                            x_sb[:, b, r0:r1, :], xv[:, b, r0:r1, :]
                        )
                    r0 = r1
                rmm = rows_per_mm
                ngrp = ROWS_PER_CORE // rmm

                def _emit_out(ps, pr, ocb):
                    if skip_out:
                        return
                    ot = opool.tile([P, rmm * W_OUT], out_dt)
                    nc.vector.tensor_copy(
                        ot[:], ps.rearrange("p a b -> p (a b)")
                    )
                    store_eng = nc.scalar if split_queues else nc.sync
                    store_eng.dma_start(
                        y[
                            ocb * P : (ocb + 1) * P,
                            pr * rmm * W_OUT : (pr + 1) * rmm * W_OUT,
                        ],
                        ot[:],
                    )

                def _mm(ps, pr, ocb, ki):
                    icb, pos = divmod(ki, 9)
                    dy, dx = divmod(pos, 3)
                    kb = 0 if same_weights else pos * ICB + icb
                    lhsT = w_sb[:, kb, ocb * P : (ocb + 1) * P]
                    rhs = x_sb[
                        :,
                        icb,
                        rmm * pr + dy : rmm * pr + dy + rmm,
                        dx : dx + W_OUT,
                    ]
                    nc.tensor.matmul(
                        ps[:, :, :],
                        lhsT,
                        rhs,
                        start=(ki == 0),
                        stop=(ki == KB - 1),
                    )

                if interleave:
                    for pr in range(ngrp):
                        psa = pspool.tile([P, rmm, W_OUT], F32, name="ps", tag="ps")
                        psb = pspool.tile([P, rmm, W_OUT], F32, name="ps", tag="ps")
                        for ki in range(KB):
                            _mm(psa, pr, 0, ki)
                            _mm(psb, pr, 1, ki)
                        _emit_out(psa, pr, 0)
                        _emit_out(psb, pr, 1)
                else:
                    for pr in range(ngrp):
                        for ocb in range(OCB):
                            ps = pspool.tile([P, rmm, W_OUT], F32, name="ps", tag="ps")
                            for ki in range(KB):
                                _mm(ps, pr, ocb, ki)
                            _emit_out(ps, pr, ocb)

            if loop_repeat == "dynamic":
                nval = nc.values_load(
                    nrep[0:1, 0:1], min_val=1, max_val=10_000_000
                )
                with tc.For_i(0, nval, 1):
                    for _rep in range(repeat):
                        _one_pass()
            elif loop_repeat > 1:
                with tc.For_i(0, loop_repeat, 1):
                    for _rep in range(repeat):
                        _one_pass()
            else:
                for _rep in range(repeat):
                    _one_pass()
    nc.compile()
    return nc


_NC_CACHE = {}


def _get_nc():
    if "nc" not in _NC_CACHE:
        _NC_CACHE["nc"] = build()
    return _NC_CACHE["nc"]


def make_in_maps(input, kernelsL, np_dt=ml_dtypes.bfloat16):
    inp = np.asarray(input, dtype=np.float32).reshape(IN_C, H, W).astype(np_dt)
    w = np.asarray(kernelsL, dtype=np.float32)
    # wT[pos*256 + ic, oc] = kernelsL[oc, ic*9 + pos]
    wT = np.ascontiguousarray(
        w.reshape(OUT_C, IN_C, 9)
        .transpose(2, 1, 0)
        .reshape(9 * IN_C, OUT_C)
        .astype(np_dt)
    )
    in_maps = []
    for c in range(NCORES):
        r0 = c * ROWS_PER_CORE
        strip = np.ascontiguousarray(inp[:, r0 : r0 + IN_ROWS, :]).reshape(
            IN_C, IN_ROWS * W
        )
        in_maps.append({"x": strip, "wT": wT})
    return in_maps


def assemble(results):
    out = np.empty((OUT_C, H_OUT, W_OUT), dtype=np.float32)
    for c in range(NCORES):
        out[:, c * ROWS_PER_CORE : (c + 1) * ROWS_PER_CORE, :] = (
            np.asarray(results[c]["y"])
            .astype(np.float32)
            .reshape(OUT_C, ROWS_PER_CORE, W_OUT)
        )
    return out.reshape(1, OUT_C, H_OUT, W_OUT)


def kernel(input, kernelsL):
    in_maps = make_in_maps(input, kernelsL)
    nc = _get_nc()
    res = run_bass_kernel_spmd(nc, in_maps, core_ids=list(range(NCORES)))
    return assemble(res.results)

